# revision 34
# baseline (speedup 1.0000x reference)
"""Trainium2 Bass kernel for KeOps multi-head latent attention.

Reference computation (B=2, N=2048, DIM=1024, LATENT=512, HEADS=16, HD=64):
    q = x @ wq * scale
    k = relu((x @ wkv[:, :D]) @ lk1) @ lk2      (folded: relu(x @ W1k) @ lk2)
    v = relu((x @ wkv[:, D:]) @ lv1) @ lv2      (folded: relu(x @ W1v) @ lv2)
    per head: e = exp(q k^T + maskbias); out = (e @ v) / (e.sum + 1e-6)
    y = out @ wout + bout

Strategy (8 cores, one SPMD NEFF):
  - tokens sharded 512/core (cores 0-3 batch0, 4-7 batch1)
  - masked keys compacted on host; each core computes k/v for P assigned
    active-key slots; one grouped AllGather [[0-3],[4-7]] shares k/v in FP8
    (e4m3).  A constant softmax shift C (folded into the per-key ACT bias,
    including the reference's +1e-6 eps key) keeps exp() inside fp8 range
    while leaving the numer/denom ratio bit-exact vs the unshifted formula.
  - attention: local 512 queries x all gathered keys, keys on partitions
    so the mask bias is a per-partition ACT bias and the denominator is a
    free ones-column in the v matmul; q/k/v/e operands fp8, fp32 PSUM
    accumulation; softmax reciprocal broadcast runs on the (otherwise
    idle) GPSIMD engine; numer/denom evacuated to SBUF immediately to
    free the PSUM banks for the next head-pair
  - weights and x in bf16; coarse rearranged-AP DMA loads (few large
    descriptors) and careful SP/ACT queue placement keep the DMA issue
    rate off the critical path
"""

import sys

sys.path.insert(0, "/opt/trn_rl_repo")
import numpy as np
import ml_dtypes
import concourse.bass as bass
import concourse.mybir as mybir
import concourse.tile as tile
from concourse import bacc
from concourse.bass_utils import run_bass_kernel_spmd

DIM, LATENT, HEADS, HD = 1024, 512, 16, 64
B, N, NC, T = 2, 2048, 8, 512
SCALE = HD ** -0.5
BF16, F32, FP8 = mybir.dt.bfloat16, mybir.dt.float32, mybir.dt.float8e4
NPBF16 = ml_dtypes.bfloat16
NEG = -10000.0

_cache: dict = {}
LAST_RESULTS = None


def _pieces(lo, hi, P):
    """Split global row range [lo,hi) into per-chunk pieces (chunk, clo, n, dst)."""
    out, r = [], lo
    while r < hi:
        c = r // P
        e = min(hi, (c + 1) * P)
        out.append((c, r - c * P, e - r, r - lo))
        r = e
    return out


def _build(NB):
    """NB = gathered key blocks of 128 per batch; P = NB*32 slots per producer."""
    P = NB * 32
    TB = (P + 127) // 128
    LK, LV = DIM * P, P * 1040
    LTOT = LK + LV

    nc = bacc.Bacc("TRN2", target_bir_lowering=False, num_devices=NC)
    xq_d = nc.dram_tensor("xq", [DIM, T], BF16, kind="ExternalInput")
    xkv_d = nc.dram_tensor("xkv", [DIM, P], BF16, kind="ExternalInput")
    wq_d = nc.dram_tensor("wq", [DIM, DIM], BF16, kind="ExternalInput")
    w1k_d = nc.dram_tensor("w1k", [DIM, LATENT], BF16, kind="ExternalInput")
    lk2_d = nc.dram_tensor("lk2", [LATENT, DIM], BF16, kind="ExternalInput")
    w1v_d = nc.dram_tensor("w1v", [DIM, LATENT], BF16, kind="ExternalInput")
    lv2_d = nc.dram_tensor("lv2", [LATENT, DIM], BF16, kind="ExternalInput")
    wout_d = nc.dram_tensor("wout", [DIM, DIM], BF16, kind="ExternalInput")
    bout_d = nc.dram_tensor("bout", [128, 8], F32, kind="ExternalInput")
    kb_d = nc.dram_tensor("kb", [128, NB], F32, kind="ExternalInput")
    y_d = nc.dram_tensor("yT", [DIM, T], F32, kind="ExternalOutput")

    from contextlib import ExitStack
    with ExitStack() as ctx:
        tc = ctx.enter_context(tile.TileContext(nc))
        pool = lambda **kw: ctx.enter_context(tc.tile_pool(**kw))
        pw = pool(name="pw", bufs=4)            # big weight tiles (transient)
        pwbig = pool(name="pwbig", bufs=2)      # wq/wout
        px = pool(name="px", bufs=2)            # xq/xkv
        ph = pool(name="ph", bufs=8)            # hk/hv latent activations
        pst = pool(name="pst", bufs=3)          # cin staging (kT, v)
        pqt = pool(name="pqt", bufs=8)          # q tiles (persist)
        pkt = pool(name="pkt", bufs=1)          # gathered kT per head-pair
        pvg = pool(name="pvg", bufs=1)          # gathered v (one big tile)
        pkb = pool(name="pkb", bufs=1)
        pe_ = pool(name="pe", bufs=3)           # exp tiles
        patt = pool(name="patt", bufs=8)        # attention outputs (persist)
        pattB = pool(name="pattB", bufs=1)
        pnmc = pool(name="pnmc", bufs=3)        # numer psum evacuation
        pd = pool(name="pd", bufs=2)            # reciprocal rows
        pbb = pool(name="pbb", bufs=2)          # broadcast reciprocal
        pbo = pool(name="pbo", bufs=1)
        posb = pool(name="posb", bufs=2)
        ps1 = pool(name="ps1", bufs=2, space="PSUM")
        pssc = pool(name="pssc", bufs=2, space="PSUM")
        psnm = pool(name="psnm", bufs=2, space="PSUM")
        dram = pool(name="dram", bufs=1, space="DRAM")
        if True:
            # ---------------- phase 1: kv path (feeds the collective) --------
            xkv_sb = px.tile([128, 8 * P], BF16, tag="xkv")
            w1k_sb = pw.tile([128, 8 * LATENT], BF16, tag="w1")
            w1v_sb = pw.tile([128, 8 * LATENT], BF16, tag="w1")
            lk2_sb = pw.tile([128, 4 * DIM], BF16, tag="w2")
            lv2_sb = pw.tile([128, 4 * DIM], BF16, tag="w2")

            for h2 in range(2):
                dd = slice(512 * h2, 512 * (h2 + 1))
                nc.sync.dma_start(
                    xkv_sb[:, 4 * P * h2:4 * P * (h2 + 1)]
                    .rearrange("p (d n) -> p d n", d=4),
                    xkv_d.ap()[dd, :].rearrange("(d p) n -> p d n", p=128))
                nc.sync.dma_start(
                    w1k_sb[:, 4 * LATENT * h2:4 * LATENT * (h2 + 1)]
                    .rearrange("p (d l) -> p d l", d=4),
                    w1k_d.ap()[dd, :].rearrange("(d p) l -> p d l", p=128))
                nc.sync.dma_start(
                    w1v_sb[:, 4 * LATENT * h2:4 * LATENT * (h2 + 1)]
                    .rearrange("p (d l) -> p d l", d=4),
                    w1v_d.ap()[dd, :].rearrange("(d p) l -> p d l", p=128))
            nc.sync.dma_start(
                lk2_sb[:].rearrange("p (l c) -> p l c", l=4),
                lk2_d.ap().rearrange("(l p) c -> p l c", p=128))
            nc.sync.dma_start(
                lv2_sb[:].rearrange("p (l c) -> p l c", l=4),
                lv2_d.ap().rearrange("(l p) c -> p l c", p=128))
            kbt = pkb.tile([128, NB], F32, tag="kb")
            nc.sync.dma_start(kbt[:], kb_d.ap())

            hk, hv = [], []
            for w_sb, dst in ((w1k_sb, hk), (w1v_sb, hv)):
                for l in range(4):
                    ps = ps1.tile([128, P], F32, tag="p1")
                    for d in range(8):
                        nc.tensor.matmul(
                            ps[:], w_sb[:, d * LATENT + 128 * l:d * LATENT + 128 * (l + 1)],
                            xkv_sb[:, d * P:(d + 1) * P],
                            start=(d == 0), stop=(d == 7))
                    h = ph.tile([128, P], BF16, tag="h")
                    nc.scalar.activation(h[:], ps[:], mybir.ActivationFunctionType.Relu)
                    dst.append(h)

            cin = dram.tile([LTOT], FP8)
            cout = dram.tile([4 * LTOT], FP8)

            # kT payload [DIM, P] feature-major, staged then one DMA
            kstage = px.tile([128, 8 * P], FP8, tag="kst")
            for cb in range(8):
                ps = ps1.tile([128, P], F32, tag="p1")
                for l in range(4):
                    nc.tensor.matmul(
                        ps[:], lk2_sb[:, l * DIM + 128 * cb:l * DIM + 128 * (cb + 1)],
                        hk[l][:], start=(l == 0), stop=(l == 3))
                with nc.allow_low_precision(reason="bf16 payload"):
                    nc.vector.tensor_copy(kstage[:, cb * P:(cb + 1) * P], ps[:])
            nc.sync.dma_start(
                cin[0:LK].rearrange("(i p s) -> p i s", i=8, p=128),
                kstage[:].rearrange("p (i s) -> p i s", i=8))

            # v payload [P, 1040] token-major, head-interleaved with ones cols
            for tb in range(TB):
                m = min(128, P - 128 * tb)
                vsb = pst.tile([128, 1040], FP8, tag="vsb")
                nc.gpsimd.memset(vsb[:m, :], 1.0)
                for ch in range(2):
                    ps = ps1.tile([128, 512], F32, tag="p1")
                    for l in range(4):
                        nc.tensor.matmul(
                            ps[:m, :], hv[l][:, 128 * tb:128 * tb + m],
                            lv2_sb[:, l * DIM + 512 * ch:l * DIM + 512 * (ch + 1)],
                            start=(l == 0), stop=(l == 3))
                    dst = vsb[0:m, 520 * ch:520 * (ch + 1)] \
                        .rearrange("p (g c) -> p g c", c=65)[:, :, 0:64]
                    src = ps[0:m, :].rearrange("p (g c) -> p g c", c=64)
                    with nc.allow_low_precision(reason="bf16 payload"):
                        nc.vector.tensor_copy(dst, src)
                nc.sync.dma_start(
                    cin[LK + 128 * tb * 1040: LK + (128 * tb + m) * 1040]
                    .rearrange("(p f) -> p f", p=m),
                    vsb[:m, :])

            xq_a = px.tile([128, 4 * T], BF16, tag="xq")
            xq_b = px.tile([128, 4 * T], BF16, tag="xq")
            nc.sync.dma_start(
                xq_a[:].rearrange("p (d n) -> p d n", d=4),
                xq_d.ap()[0:512, :].rearrange("(d p) n -> p d n", p=128))
            nc.sync.dma_start(
                xq_b[:].rearrange("p (d n) -> p d n", d=4),
                xq_d.ap()[512:1024, :].rearrange("(d p) n -> p d n", p=128))
            wq_a = pwbig.tile([128, 4 * DIM], BF16, tag="wq")
            wq_b = pwbig.tile([128, 4 * DIM], BF16, tag="wq")
            nc.sync.dma_start(
                wq_a[:].rearrange("p (d c) -> p d c", d=4),
                wq_d.ap()[0:512, :].rearrange("(d p) c -> p d c", p=128))
            nc.sync.dma_start(
                wq_b[:].rearrange("p (d c) -> p d c", d=4),
                wq_d.ap()[512:1024, :].rearrange("(d p) c -> p d c", p=128))

            nc.gpsimd.collective_compute(
                "AllGather", mybir.AluOpType.bypass,
                replica_groups=[[0, 1, 2, 3], [4, 5, 6, 7]],
                ins=[cin.opt()], outs=[cout.opt()],
            )

            # ---------------- q path (overlaps the collective) ---------------
            qt = []
            for cb in range(8):
                ps = ps1.tile([128, T], F32, tag="p1")
                for d in range(8):
                    xq_h = xq_a if d < 4 else xq_b
                    wq_h = wq_a if d < 4 else wq_b
                    nc.tensor.matmul(
                        ps[:], wq_h[:, (d % 4) * DIM + 128 * cb:(d % 4) * DIM + 128 * (cb + 1)],
                        xq_h[:, (d % 4) * T:(d % 4 + 1) * T],
                        start=(d == 0), stop=(d == 7))
                q = pqt.tile([128, T], FP8, tag="qt")
                with nc.allow_low_precision(reason="bf16 q"):
                    nc.vector.tensor_copy(q[:], ps[:])
                qt.append(q)

            # ---------------- gathered kv loads ------------------------------
            cout2 = cout[:].rearrange("(t x) -> t x", t=4)
            kt_all = pkt.tile([128, 8 * NB * 128], FP8, tag="ktg")
            for t4 in range(4):
                nc.sync.dma_start(
                    kt_all[:].rearrange("p (i t s) -> p t i s", i=8, t=4)[:, t4],
                    cout2[t4:t4 + 1, 0:LK]
                    .rearrange("t (i p s) -> p (t i) s", i=8, p=128))
            kt = [kt_all[:, i * NB * 128:(i + 1) * NB * 128] for i in range(8)]
            vg = pvg.tile([128, NB * 1040], FP8, tag="vg")
            for j in range(NB):
                for (c, clo, n, dst) in _pieces(128 * j, 128 * (j + 1), P):
                    nc.sync.dma_start(
                        vg[dst:dst + n, 1040 * j:1040 * (j + 1)],
                        cout[c * LTOT + LK + clo * 1040:
                             c * LTOT + LK + (clo + n) * 1040]
                        .rearrange("(p f) -> p f", p=n))

            boutt = pbo.tile([128, 8], F32, tag="bo")
            nc.sync.dma_start(boutt[:], bout_d.ap())
            wout_a = pwbig.tile([128, 4 * DIM], BF16, tag="wo")
            wout_b = pwbig.tile([128, 4 * DIM], BF16, tag="wo")
            nc.sync.dma_start(
                wout_a[:].rearrange("p (r c) -> p r c", r=4),
                wout_d.ap()[0:512, :].rearrange("(r p) c -> p r c", p=128))
            nc.sync.dma_start(
                wout_b[:].rearrange("p (r c) -> p r c", r=4),
                wout_d.ap()[512:1024, :].rearrange("(r p) c -> p r c", p=128))

            # ---------------- attention -------------------------------------
            att = []
            Exp = mybir.ActivationFunctionType.Exp
            for i in range(8):
                nA = psnm.tile([65, 512], F32, tag="nm")
                nB = psnm.tile([65, 512], F32, tag="nm")
                for j in range(NB):
                    ktj = kt[i][:, 128 * j:128 * (j + 1)]
                    sc = pssc.tile([128, 1024], F32, tag="sc")
                    nc.tensor.matmul(sc[:, 0:512], ktj[0:64, :], qt[i][0:64, :],
                                     start=True, stop=True)
                    nc.tensor.matmul(sc[:, 512:1024], ktj[64:128, :], qt[i][64:128, :],
                                     start=True, stop=True)
                    e = pe_.tile([128, 1024], FP8, tag="e")
                    with nc.allow_low_precision(reason="bf16 softmax weights"):
                        nc.scalar.activation(e[:], sc[:], Exp, bias=kbt[:, j:j + 1])
                    nc.tensor.matmul(nA[:], vg[:, 1040 * j + 130 * i:1040 * j + 130 * i + 65],
                                     e[:, 0:512], start=(j == 0), stop=(j == NB - 1))
                    nc.tensor.matmul(nB[:], vg[:, 1040 * j + 130 * i + 65:1040 * j + 130 * i + 130],
                                     e[:, 512:1024], start=(j == 0), stop=(j == NB - 1))
                ap_t = patt.tile([128, 512], BF16, tag="att")
                aB = pattB.tile([64, 512], BF16, tag="attB")
                if i < 7:
                    # evacuate PSUM first so the banks free for the next pair
                    for half, (nm, outap) in enumerate(((nA, ap_t[0:64, :]), (nB, aB[:]))):
                        nmc = pnmc.tile([65, 512], BF16, tag="nmc")
                        d_sb = pd.tile([1, 512], BF16, tag="d")
                        bb = pbb.tile([64, 512], BF16, tag="bb")
                        with nc.allow_low_precision(reason="bf16 softmax normalize"):
                            nc.vector.tensor_copy(nmc[:], nm[:])
                            nc.vector.reciprocal(d_sb[:], nmc[64:65, :])
                            nc.gpsimd.partition_broadcast(bb[:], d_sb[:])
                            nc.vector.tensor_mul(outap, nmc[0:64, :], bb[:])
                else:
                    # last pair: nothing needs the banks next — divide straight
                    # from PSUM with A/B interleaved to shorten the tail chain
                    d_a = pd.tile([1, 512], BF16, tag="d")
                    d_b = pd.tile([1, 512], BF16, tag="d", name="d_b")
                    bb_a = pbb.tile([64, 512], BF16, tag="bb")
                    bb_b = pbb.tile([64, 512], BF16, tag="bb", name="bb_b")
                    with nc.allow_low_precision(reason="bf16 softmax normalize"):
                        nc.vector.reciprocal(d_a[:], nA[64:65, :])
                        nc.vector.reciprocal(d_b[:], nB[64:65, :])
                        nc.gpsimd.partition_broadcast(bb_a[:], d_a[:])
                        nc.gpsimd.partition_broadcast(bb_b[:], d_b[:])
                        nc.vector.tensor_mul(ap_t[0:64, :], nA[0:64, :], bb_a[:])
                        nc.vector.tensor_mul(aB[:], nB[0:64, :], bb_b[:])
                nc.sync.dma_start(ap_t[64:128, :], aB[:])
                att.append(ap_t)

            # ---------------- output projection ------------------------------
            for cb in range(8):
                ps = ps1.tile([128, T], F32, tag="p1")
                for i in range(8):
                    wo_h = wout_a if i < 4 else wout_b
                    nc.tensor.matmul(
                        ps[:], wo_h[:, (i % 4) * DIM + 128 * cb:(i % 4) * DIM + 128 * (cb + 1)],
                        att[i][:], start=(i == 0), stop=(i == 7))
                osb = posb.tile([128, T], F32, tag="osb")
                nc.vector.tensor_scalar_add(osb[:], ps[:], boutt[:, cb:cb + 1])
                nc.sync.dma_start(y_d.ap()[128 * cb:128 * (cb + 1), :], osb[:])

    nc.compile()
    return nc


def kernel(x, mask, wq, wkv, lk1, lk2, lv1, lv2, wout, bout, **kw):
    global LAST_RESULTS
    x = np.asarray(x, np.float32)
    mask = np.asarray(mask)
    wq_s = (np.asarray(wq, np.float32) * np.float32(SCALE)).astype(NPBF16)
    w1k = (np.asarray(wkv[:, :DIM], np.float32) @ np.asarray(lk1, np.float32)).astype(NPBF16)
    w1v = (np.asarray(wkv[:, DIM:], np.float32) @ np.asarray(lv1, np.float32)).astype(NPBF16)
    lk2 = np.ascontiguousarray(np.asarray(lk2, np.float32)).astype(NPBF16)
    lv2 = np.ascontiguousarray(np.asarray(lv2, np.float32)).astype(NPBF16)
    wout = np.ascontiguousarray(np.asarray(wout, np.float32)).astype(NPBF16)
    bout2 = np.ascontiguousarray(np.asarray(bout, np.float32).reshape(8, 128).T)

    x_flat = x.reshape(B * N, DIM)
    act = [np.nonzero(np.asarray(mask[b]) == 1)[0] for b in range(B)]
    A = [len(a) for a in act]
    NB = max(1, (max(A) + 1 + 127) // 128)
    P = NB * 32

    # per-batch kv slot -> global token (or -1 pad) and key bias
    slot_tok = np.full((B, NB * 128), -1, np.int64)
    kb = np.full((B, NB * 128), NEG, np.float32)
    # constant softmax shift: e' = exp(s - C) keeps the numer/denom ratio
    # exact (the eps key shifts too) while keeping e' within fp8e4m3 range
    C = 3.5
    for b in range(B):
        slot_tok[b, :A[b]] = b * N + act[b]
        kb[b, :A[b]] = -C
        kb[b, A[b]] = np.log(1e-6) - C  # reference's denom + 1e-6
    # [slot] -> [128, NB] with slot = 128*j + p
    kb2 = np.ascontiguousarray(kb.reshape(B, NB, 128).transpose(0, 2, 1))

    if NB not in _cache:
        _cache[NB] = _build(NB)
    nc = _cache[NB]

    in_maps = []
    for c in range(NC):
        b = c // 4
        toks = slot_tok[b, (c % 4) * P:(c % 4 + 1) * P]
        xkv = np.zeros((DIM, P), NPBF16)
        real = toks >= 0
        xkv[:, real] = x_flat[toks[real]].T.astype(NPBF16)
        in_maps.append({
            "xq": np.ascontiguousarray(x_flat[c * T:(c + 1) * T].T.astype(NPBF16)),
            "xkv": xkv,
            "wq": wq_s, "w1k": w1k, "lk2": lk2, "w1v": w1v, "lv2": lv2,
            "wout": wout, "bout": bout2, "kb": kb2[b],
        })

    res = run_bass_kernel_spmd(nc, in_maps, core_ids=list(range(NC)))
    LAST_RESULTS = res
    y = np.empty((B * N, DIM), np.float32)
    for c in range(NC):
        y[c * T:(c + 1) * T] = res.results[c]["yT"].T
    return y.reshape(B, N, DIM)
